# revision 33
# baseline (speedup 1.0000x reference)
"""Trainium2 Bass kernel for nn_AttentionLayer (B=8, N=2048, D=512).

Sharding: data-parallel over batch - one batch element per NeuronCore (8 cores),
no collectives.

Per-core pipeline (x_b [2048, 512]), chunk-major over 4 q-chunks of 512 rows.
Numerics: fp8e4m3 DoubleRow matmuls for the expand / attention / project GEMMs
with an fp8 weight-residual (and activation-residual for expand) to recover
~bf16 accuracy; bf16 for logits q/k and the PE transposes.

  1. LayerNorm via DVE bn_stats/aggregate; normalize -> bf16 nxt; PE-transpose
     (bf16) -> PSUM; ACT copy -> fp8 nx8T; DVE sub -> fp8 residual nxr8T.
  2. Expand GEMM (h = nx @ expand, cols x32) as 3 fp8-DR passes:
     nx8@W8 + nx8@WR8 + nxr8@W8 (contraction 512 = 2 pair-matmuls per pass).
     T-form for q/k/local-linear/local-pregelu; natural for v-linear/v-pregelu.
     q copy folds 1/(32*sqrt(dk)), k copy folds 1/32 -> bf16; gelu ACT scale
     1/32 -> bf16; DVE mult psum*gelt -> fp8 gated (x32, in [-240,240]).
  3. Logits bf16 (q@k), sigmoid-causal mask added via identity-stationary
     matmul into the same PSUM for near-diagonal tiles only (sigmoid(d+4)~=0
     beyond ~20 positions); ACT exp PSUM -> fp8 e tiles (pair layout).
  4. attn = v8.T @ e8 in fp8-DR over k-tile pairs, 4 d-block passes; denom via
     ones8-DR matmul; DVE mult by partition-broadcast reciprocal -> fp8 attn
     (x32), matching the local-gated scale.
  5. Project in fp8-DR (stationary gated/attn pairs, moving P8 x64 pairs,
     + optional P-residual pass); ACT copy scale 1/2048 -> SBUF; GPSIMD add
     residual x; DMA out.
"""

import numpy as np

import concourse.bass as bass
import concourse.mybir as mybir
import concourse.tile as tile
import concourse.bass_utils as bass_utils
from concourse.masks import make_identity
from concourse.vector_clock import ScopedClock

F32 = mybir.dt.float32
BF16 = mybir.dt.bfloat16
F8 = mybir.dt.float8e4
AF = mybir.ActivationFunctionType
ALU = mybir.AluOpType
DR = mybir.MatmulPerfMode.DoubleRow

B = 8
N = 2048
D = 512
QK = 64
ED = 1024
LN_EPS = 1e-5
NCH = 4            # q chunks of 512
CH = N // NCH      # 512
WE = 32.0          # expand weight scale (gated carries x32)
WP = 64.0          # project weight scale
PROJ_RESIDUAL = True

# T-part featblocks: (col0 in expand, width, sbuf byte offset in wt8)
_TBLOCKS = []
_off = 0
for c0, mf in ([(0, 64), (64, 64)]
               + [(128 + 128 * j, 128) for j in range(4)]
               + [(1152 + 128 * j, 128) for j in range(4)]):
    _TBLOCKS.append((c0, mf, _off))
    _off += 4 * mf
WT_COLS = _off          # 4608
WN_COLS = 4096
WPR_COLS = 4096


# ----------------------------------------------------------------------------
# Workaround for the walrus build in this container: CTRL-class instructions
# (Drain/NoOp) support only ONE sync-wait command. Split multi-wait
# instructions by hoisting extra waits onto preceding same-engine NOPs.
# ----------------------------------------------------------------------------
_SPLIT_LIMIT = 1
_patched = [False]


def _apply_patches():
    if _patched[0]:
        return
    _patched[0] = True

    orig_add = tile.TileContext._add_instruction
    ctr = [0]

    def _split_add(self, inst):
        si = inst.sync_info
        if (si is not None and si.on_wait and len(si.on_wait) > _SPLIT_LIMIT
                and inst.engine != mybir.EngineType.Unassigned):
            waits = list(si.on_wait)
            for w in waits[:-_SPLIT_LIMIT]:
                ctr[0] += 1
                nop = mybir.InstNoOp(name=f"I-waitsplit-{ctr[0]}", ins=[], outs=[])
                nop.engine = inst.engine
                nop.sync_info = mybir.SyncInfo(on_wait=[w], on_update=[])
                orig_add(self, nop)
            si.on_wait = waits[-_SPLIT_LIMIT:]
        orig_add(self, inst)

    tile.TileContext._add_instruction = _split_add

    def _patched_drain_and_barrier(self, tick_clock, wait_clock):
        nc = self.nc
        drain_inst = nc.sync.drain()
        wait_clock.add_sem_waits(
            drain_inst.ins, ScopedClock({None: tick_clock.global_clock})
        )
        si = drain_inst.ins.sync_info
        if si is not None and si.on_wait and len(si.on_wait) > _SPLIT_LIMIT:
            waits = list(si.on_wait)
            si.on_wait = waits[:_SPLIT_LIMIT]
            for w in waits[_SPLIT_LIMIT:]:
                d2 = nc.sync.drain()
                s2 = d2.ins.sync_info
                if s2 is None:
                    d2.ins.sync_info = mybir.SyncInfo(on_wait=[w], on_update=[])
                else:
                    s2.on_wait = [w]
        nc.all_engine_barrier()
        popped = nc._tile_sem_poison_stack.pop()
        assert popped is self._sem_poison
        nc.clear_and_free_semaphores(list(self.sems.allocated().values()))
        nc.all_engine_barrier()

    tile.TileContext._drain_and_barrier = _patched_drain_and_barrier


def _pairs(ap1024):
    """View a [128, 2*L] AP as the DoubleRow pair layout [128, 2, L]."""
    L = ap1024.shape[1] // 2
    return ap1024.rearrange("p (i m) -> p i m", i=2)


def _emit(nc, tc):
    x = nc.dram_tensor("x", [N, D], F32, kind="ExternalInput").ap()
    wt8d = nc.dram_tensor("wt8", [128, WT_COLS], F8, kind="ExternalInput").ap()
    wtr8d = nc.dram_tensor("wtr8", [128, WT_COLS], F8, kind="ExternalInput").ap()
    wn8d = nc.dram_tensor("wn8", [128, WN_COLS], F8, kind="ExternalInput").ap()
    wnr8d = nc.dram_tensor("wnr8", [128, WN_COLS], F8, kind="ExternalInput").ap()
    wp8d = nc.dram_tensor("wp8", [128, WPR_COLS], F8, kind="ExternalInput").ap()
    if PROJ_RESIDUAL:
        wpr8d = nc.dram_tensor("wpr8", [128, WPR_COLS], F8,
                               kind="ExternalInput").ap()
    maskd = nc.dram_tensor("maskT", [N, N], BF16, kind="ExternalInput").ap()
    y = nc.dram_tensor("y", [N, D], F32, kind="ExternalOutput").ap()

    from contextlib import ExitStack
    with ExitStack() as _ctx:
        def _pool(name, bufs, space="SBUF"):
            return _ctx.enter_context(
                tc.tile_pool(name=name, bufs=bufs, space=space))

        constp = _pool("constp", 1)
        wgt = _pool("wgt", 1)
        xp = _pool("xp", 1)
        nx8p = _pool("nx8p", 1)
        stp = _pool("stp", 4)
        nxtp = _pool("nxtp", 3)
        qp = _pool("qp", 1)
        kp_ = _pool("kp", 1)
        geltp = _pool("geltp", 2)
        glt8p = _pool("glt8p", 1)
        v8p = _pool("v8p", 1)
        e2p = _pool("e2p", 1)
        a8p = _pool("a8p", 1)
        mkp = _pool("mkp", 5)
        denp = _pool("denp", 2)
        obp = _pool("obp", 3)
        yp = _pool("yp", 3)
        psE = _pool("psE", 3, space="PSUM")
        psT = _pool("psT", 1, space="PSUM")
        psL = _pool("psL", 2, space="PSUM")
        psD = _pool("psD", 1, space="PSUM")

        identf = constp.tile([128, 128], F32, tag="identf")
        make_identity(nc, identf)
        identb = constp.tile([128, 128], BF16, tag="identb")
        nc.vector.tensor_copy(identb, identf)
        epst = constp.tile([128, 1], F32, tag="epst")
        nc.vector.memset(epst, LN_EPS)
        ones8 = constp.tile([128, 32], F8, tag="ones8")
        nc.vector.memset(ones8, 1.0)
        onesrow = constp.tile([1, 128], BF16, tag="onesrow")
        nc.vector.memset(onesrow, 1.0)

        # persistent tiles
        k_all = [kp_.tile([64, CH], BF16, tag=f"k{c}", name=f"k{c}")
                 for c in range(NCH)]
        q_all = [qp.tile([64, CH], BF16, tag=f"q{c}", name=f"q{c}")
                 for c in range(NCH)]
        v8 = [v8p.tile([128, 1024], F8, tag=f"v{p}", name=f"v{p}")
              for p in range(8)]
        nx8s = [nx8p.tile([128, 4 * CH], F8, tag=f"nx8_{c}", name=f"nx8_{c}")
                for c in range(NCH)]
        nxr8s = ([nx8p.tile([128, 4 * CH], F8, tag=f"nxr8_{c}",
                            name=f"nxr8_{c}") for c in range(NCH)]
                 if EXPAND_PASSES >= 3 else [None] * NCH)
        glt8s = [[glt8p.tile([128, 1024], F8, tag=f"glt{c}_{jp}",
                             name=f"glt{c}_{jp}") for jp in range(2)]
                 for c in range(NCH)]
        attn8s = [[a8p.tile([128, 1024], F8, tag=f"a{c}_{jp}",
                            name=f"a{c}_{jp}") for jp in range(2)]
                  for c in range(NCH)]
        # ------------ phase A+B: x DMA, LN, transpose -> fp8 nxT + residual -
        # per-tile Sqrt is fine for the ACT table: every sqrt precedes every
        # gelu in the ACT queue, and Copy lives in all tables.
        # Weights are interleaved on SP after chunk-0's x tiles so the first
        # expand matmuls aren't starved.
        wgt_tiles = {}

        def _emit_weight_dmas():
            specs = [("wt", WT_COLS, wt8d), ("wtr", WT_COLS, wtr8d),
                     ("wn", WN_COLS, wn8d), ("wnr", WN_COLS, wnr8d),
                     ("wp", WPR_COLS, wp8d)]
            if PROJ_RESIDUAL:
                specs.append(("wpr", WPR_COLS, wpr8d))
            for tag, cols, dram in specs:
                tl = wgt.tile([128, cols], F8, tag=tag, name=f"w_{tag}")
                nc.sync.dma_start(tl, dram)
                wgt_tiles[tag] = tl

        # chunk 0 gets per-tile sqrts (all before the first gelu); chunks 1-3
        # share ONE batched sqrt+reciprocal so only 2 ACT table switches land
        # mid-gelu-phase instead of 12.
        mv32 = constp.tile([128, 32], F32, tag="mv32")
        rstd12 = constp.tile([128, 12], F32, tag="rstd12")
        x_tiles = []
        mus, rstds = [], []
        for r in range(16):
            xt = xp.tile([128, D], F32, tag=f"x{r}", name=f"x_{r}")
            nc.sync.dma_start(xt, x[r * 128:(r + 1) * 128, :])
            x_tiles.append(xt)
            if r == 3:
                _emit_weight_dmas()
            st6 = stp.tile([128, 6], F32, tag="st6")
            nc.vector.bn_stats(st6, xt)
            if r < 4:
                st2 = stp.tile([128, 2], F32, tag="st2", bufs=4)
                nc.vector.bn_aggr(st2, st6)
                std = stp.tile([128, 1], F32, tag="std")
                nc.scalar.activation(std, st2[:, 1:2], AF.Sqrt, bias=epst)
                rstd = stp.tile([128, 1], F32, tag="rstd", bufs=4)
                nc.vector.reciprocal(rstd, std)
                mus.append(st2[:, 0:1])
                rstds.append(rstd)
            else:
                nc.vector.bn_aggr(mv32[:, 2 * r:2 * r + 2], st6)
                mus.append(mv32[:, 2 * r:2 * r + 1])
                rstds.append(rstd12[:, r - 4:r - 3])
        std12 = stp.tile([128, 12], F32, tag="std12")
        nc.scalar.activation(
            std12,
            mv32.rearrange("p (r two) -> p two r", two=2)[:, 1, 4:16],
            AF.Sqrt, bias=epst)
        nc.vector.reciprocal(rstd12, std12)

        for r in range(16):
            c, t = r // 4, r % 4
            nxt = nxtp.tile([128, D], BF16, tag="nxt")
            nc.vector.tensor_scalar(nxt, x_tiles[r], mus[r], rstds[r],
                                    op0=ALU.subtract, op1=ALU.mult)
            tp = psT.tile([128, 512], BF16, tag="tp")
            for j in range(4):
                nc.tensor.matmul(tp[:, j * 128:(j + 1) * 128],
                                 nxt[:, j * 128:(j + 1) * 128], identb,
                                 is_transpose=True, skip_group_check=True)
            # tp holds 4 transposed d-blocks; scatter to nx8 cols j*512+t*128
            tp3 = tp.rearrange("p (j t) -> p j t", j=4)
            o3 = nx8s[c].rearrange("p (j t) -> p j t", j=4)[:, :,
                                                           t * 128:(t + 1) * 128]
            nc.scalar.activation(o3, tp3, AF.Copy)
            if EXPAND_PASSES >= 3:
                r3 = nxr8s[c].rearrange("p (j t) -> p j t", j=4)[:, :,
                                                                t * 128:
                                                                (t + 1) * 128]
                nc.vector.tensor_sub(r3, tp3, o3)

        wt_sb, wtr_sb = wgt_tiles["wt"], wgt_tiles["wtr"]
        wn_sb, wnr_sb = wgt_tiles["wn"], wgt_tiles["wnr"]
        wp_sb = wgt_tiles["wp"]
        if PROJ_RESIDUAL:
            wpr_sb = wgt_tiles["wpr"]

        # ------------ phase C: expand GEMMs (all chunks; ACT = gelu+copies) -
        # q/k blocks are computed LAST so that no attention logit (and hence
        # no Exp) becomes schedulable before all Gelus retire: this keeps the
        # ACT function-table resident (a table switch costs 1283 ns).
        def _nxpairs(c):
            return ([_pairs(nx8s[c][:, kp * 1024:(kp + 1) * 1024])
                     for kp in range(2)],
                    [_pairs(nxr8s[c][:, kp * 1024:(kp + 1) * 1024])
                     for kp in range(2)] if EXPAND_PASSES >= 3
                    else [None, None])

        def dr3(ps_ap, sta_w, sta_r, mov8, movr, first, last):
            three = EXPAND_PASSES >= 3
            nc.tensor.matmul(ps_ap, sta_w, mov8, start=first, stop=False,
                             perf_mode=DR)
            nc.tensor.matmul(ps_ap, sta_r, mov8, start=False,
                             stop=(last and not three), perf_mode=DR)
            if three:
                nc.tensor.matmul(ps_ap, sta_w, movr, start=False, stop=last,
                                 perf_mode=DR)

        def t_block(c, bi):
            nx8pair, nxr8pair = _nxpairs(c)
            c0, mf, off = _TBLOCKS[bi]
            pf = psE.tile([128, 512], F32, tag="ps")
            for kp in range(2):
                sw = _pairs(wt_sb[:, off + kp * 2 * mf:
                                  off + (kp + 1) * 2 * mf])
                sr = _pairs(wtr_sb[:, off + kp * 2 * mf:
                                   off + (kp + 1) * 2 * mf])
                dr3(pf[:mf], sw, sr, nx8pair[kp], nxr8pair[kp],
                    kp == 0, kp == 1)
            return pf

        for c in range(NCH):
            nx8pair, nxr8pair = _nxpairs(c)
            for j in range(4):
                pl = t_block(c, 2 + j)
                pg = t_block(c, 6 + j)
                gelt = geltp.tile([128, CH], BF16, tag="gelt")
                nc.scalar.activation(gelt, pg, AF.Gelu, scale=1.0 / WE)
                nc.vector.tensor_mul(
                    glt8s[c][j // 2][:, (j % 2) * 512:(j % 2 + 1) * 512],
                    pl, gelt)

            # natural part -> v8
            for t in range(4):
                r = 4 * c + t
                pl = psE.tile([128, 512], F32, tag="ps")
                pg = psE.tile([128, 512], F32, tag="ps")
                for dst, base in ((pl, 0), (pg, 1024)):
                    for kp in range(2):
                        sta8 = nx8pair[kp][:, :, t * 128:(t + 1) * 128]
                        star = (nxr8pair[kp][:, :, t * 128:(t + 1) * 128]
                                if EXPAND_PASSES >= 3 else None)
                        mw = _pairs(wn_sb[:, kp * 2048 + base:
                                          kp * 2048 + base + 1024])
                        mr = _pairs(wnr_sb[:, kp * 2048 + base:
                                           kp * 2048 + base + 1024])
                        three = EXPAND_PASSES >= 3
                        nc.tensor.matmul(dst, sta8, mw, start=(kp == 0),
                                         stop=False, perf_mode=DR)
                        nc.tensor.matmul(dst, sta8, mr, start=False,
                                         stop=(kp == 1 and not three),
                                         perf_mode=DR)
                        if three:
                            nc.tensor.matmul(dst, star, mw, start=False,
                                             stop=(kp == 1), perf_mode=DR)
                vg = geltp.tile([128, D], BF16, tag="vg")
                nc.scalar.activation(vg, pg, AF.Gelu, scale=1.0 / WE)
                nc.vector.tensor_mul(
                    v8[r // 2][:, (r % 2) * 512:(r % 2 + 1) * 512], pl, vg)

        # q/k for all chunks, after every gelu is emitted (see note above);
        # on DVE (tensor_scalar) to keep ACT free for gelu/exp
        for c in range(NCH):
            pf = t_block(c, 0)
            nc.vector.tensor_scalar_mul(q_all[c], pf[:64], 1.0 / (WE * 8.0))
            pf = t_block(c, 1)
            nc.vector.tensor_scalar_mul(k_all[c], pf[:64], 1.0 / WE)

        # ------------ phase D+E: attention then project, per chunk ----------
        # The attention d-pass accumulators and the project accumulators share
        # the psE ring (phases are disjoint in time), freeing a PSUM bank for
        # deeper expand pipelining. Project(c) is emitted right after
        # attention(c) so its matmuls fill PE slack while the next chunk's
        # exps run on ACT.
        for c in range(NCH):
            npair = 2 * c + 2
            e2 = [e2p.tile([128, 1024], F8, tag=f"e{kp}", name=f"e{kp}_{c}")
                  for kp in range(npair)]
            den16 = psD.tile([16, 512], F32, tag="den")
            for kp in range(npair):
                for sub in range(2):
                    kt = 2 * kp + sub
                    near = kt >= 4 * c - 1
                    lg = psL.tile([128, 512], F32, tag="lg")
                    nc.tensor.matmul(lg,
                                     k_all[kt // 4][:, (kt % 4) * 128:
                                                    (kt % 4 + 1) * 128],
                                     q_all[c], start=True, stop=not near)
                    if near:
                        mk = mkp.tile([128, CH], BF16, tag="mk")
                        nc.sync.dma_start(
                            mk, maskd[kt * 128:(kt + 1) * 128,
                                      c * CH:(c + 1) * CH])
                        nc.tensor.matmul(lg, identb, mk, start=False,
                                         stop=True)
                    nc.scalar.activation(
                        e2[kp][:, sub * 512:(sub + 1) * 512], lg, AF.Exp)
                nc.tensor.matmul(den16, _pairs(ones8), _pairs(e2[kp]),
                                 start=(kp == 0), stop=(kp == npair - 1),
                                 perf_mode=DR)

            # reciprocal -> bf16 -> PE ones-matmul broadcast into PSUM (the
            # DVE InstReciprocal is the accurate variant, and bf16 rounding
            # dwarfs its error anyway)
            r1b = denp.tile([1, 512], BF16, tag="r1b")
            nc.vector.reciprocal(r1b, den16[0:1, :])
            recip_ps = psD.tile([128, 512], F32, tag="rps")
            nc.tensor.matmul(recip_ps, onesrow, r1b)
            recip_bc = denp.tile([128, 512], BF16, tag="rbc")
            nc.scalar.activation(recip_bc, recip_ps, AF.Copy)

            for j in range(4):
                pa = psE.tile([128, 512], F32, tag="ps")
                for kp in range(npair):
                    nc.tensor.matmul(
                        pa, _pairs(v8[kp])[:, :, j * 128:(j + 1) * 128],
                        _pairs(e2[kp]), start=(kp == 0),
                        stop=(kp == npair - 1), perf_mode=DR)
                nc.vector.tensor_mul(
                    attn8s[c][j // 2][:, (j % 2) * 512:(j % 2 + 1) * 512],
                    pa, recip_bc)

            for t in range(4):
                r = 4 * c + t
                po = psE.tile([128, 512], F32, tag="ps")
                nmm = 8 if PROJ_RESIDUAL else 4
                i = 0
                for sta_src, base in ((glt8s[c], 0), (attn8s[c], 2048)):
                    for jp in range(2):
                        sta = _pairs(sta_src[jp])[:, :, t * 128:(t + 1) * 128]
                        mv = _pairs(wp_sb[:, base + jp * 1024:
                                          base + (jp + 1) * 1024])
                        nc.tensor.matmul(po, sta, mv, start=(i == 0),
                                         stop=(i == nmm - 1), perf_mode=DR)
                        i += 1
                        if PROJ_RESIDUAL:
                            mvr = _pairs(wpr_sb[:, base + jp * 1024:
                                                base + (jp + 1) * 1024])
                            nc.tensor.matmul(po, sta, mvr, start=False,
                                             stop=(i == nmm - 1),
                                             perf_mode=DR)
                            i += 1
                ob = obp.tile([128, D], F32, tag="ob")
                nc.vector.tensor_scalar_mul(ob, po, 1.0 / (WE * WP))
                yt = yp.tile([128, D], F32, tag="yt")
                nc.gpsimd.tensor_add(yt, ob, x_tiles[r])
                nc.sync.dma_start(y[r * 128:(r + 1) * 128, :], yt)


_cached = {}


def _build(loop=None):
    import os

    if loop is None:
        loop = int(os.environ.get("ATTN_LOOP", "0"))
    key = ("nc", loop)
    if key in _cached:
        return _cached[key]
    _apply_patches()
    nc = bass.Bass("TRN2", target_bir_lowering=False, debug=False)
    with nc.allow_low_precision("fp8/bf16 kernel with residual correction"):
        with tile.TileContext(nc) as tc:
            if loop > 1:
                with tc.For_i(0, loop, 1):
                    _emit(nc, tc)
            else:
                _emit(nc, tc)
    _cached[key] = nc
    return nc


def _q8(a):
    import ml_dtypes
    return np.clip(a, -240.0, 240.0).astype(ml_dtypes.float8_e4m3)


def _pack_pairs_T(E8, blocks):
    """T-part stationary: per (block, kp): [128, 2, mf] -> [128, 4*mf]."""
    segs = []
    for c0, mf, _ in blocks:
        for kp in range(2):
            t = np.empty((128, 2, mf), dtype=E8.dtype)
            for i in range(2):
                t[:, i, :] = E8[(2 * kp + i) * 128:(2 * kp + i + 1) * 128,
                                c0:c0 + mf]
            segs.append(t.reshape(128, 2 * mf))
    return np.concatenate(segs, axis=1)


def _pack_pairs_mov(M8, row_pairs, col0, ncol):
    """Moving pairs [128, 2, ncol] for given row pair index."""
    t = np.empty((128, 2, ncol), dtype=M8.dtype)
    for i in range(2):
        r0 = (2 * row_pairs + i) * 128
        t[:, i, :] = M8[r0:r0 + 128, col0:col0 + ncol]
    return t.reshape(128, 2 * ncol)


def _host_prep(expand, project, position_bias_mult):
    import ml_dtypes

    E = np.asarray(expand, dtype=np.float32) * WE
    E8 = _q8(E)
    ER8 = _q8(E - E8.astype(np.float32))
    P = np.asarray(project, dtype=np.float32) * WP
    P8 = _q8(P)
    PR8 = _q8(P - P8.astype(np.float32))

    wt8 = _pack_pairs_T(E8, _TBLOCKS)
    wtr8 = _pack_pairs_T(ER8, _TBLOCKS)
    # natural moving: kp-major, [lin-v 1024][pre-v 1024] per kp
    wn8 = np.concatenate(
        [np.concatenate([_pack_pairs_mov(E8, kp, 640, 512),
                         _pack_pairs_mov(E8, kp, 1664, 512)], axis=1)
         for kp in range(2)], axis=1)
    wnr8 = np.concatenate(
        [np.concatenate([_pack_pairs_mov(ER8, kp, 640, 512),
                         _pack_pairs_mov(ER8, kp, 1664, 512)], axis=1)
         for kp in range(2)], axis=1)
    wp8 = np.concatenate([_pack_pairs_mov(P8, fp, 0, 512)
                          for fp in range(4)], axis=1)
    wpr8 = np.concatenate([_pack_pairs_mov(PR8, fp, 0, 512)
                           for fp in range(4)], axis=1)

    pbm = np.float64(position_bias_mult)
    idx = np.arange(N, dtype=np.float64)
    kk = idx[:, None]
    qq = idx[None, :]
    d = kk - qq
    with np.errstate(over="ignore"):
        m = 1.0 / (1.0 + np.exp(-(d + pbm)))
    maskT = np.where(kk <= qq, m, -10000.0).astype(ml_dtypes.bfloat16)
    return wt8, wtr8, wn8, wnr8, wp8, wpr8, maskT


def kernel(x, expand, project, position_bias_mult):
    import os

    nc = _build()
    wt8, wtr8, wn8, wnr8, wp8, wpr8, maskT = _host_prep(
        expand, project, position_bias_mult)
    xs = np.ascontiguousarray(np.asarray(x, dtype=np.float32))
    in_maps = []
    for b in range(B):
        m = {"x": xs[b], "wt8": wt8, "wtr8": wtr8, "wn8": wn8, "wnr8": wnr8,
             "wp8": wp8, "maskT": maskT}
        if PROJ_RESIDUAL:
            m["wpr8"] = wpr8
        in_maps.append(m)
    trace = bool(int(os.environ.get("ATTN_TRACE", "0")))
    res = bass_utils.run_bass_kernel_spmd(
        nc, in_maps, core_ids=list(range(B)), trace=trace)
    _cached["exec_time_ns"] = res.exec_time_ns
    return np.stack([r["y"] for r in res.results], axis=0)
